# revision 2
# baseline (speedup 1.0000x reference)
"""Trainium2 Bass kernel for the DF time-loop module (nn_DfOpTimeLoop).

Strategy
--------
Shard the T=60000 time axis across 8 NeuronCores (7500 frames each).
All the reference's quirky edge behavior folds into a host-built halo
buffer H (frames 0/1 swapped, two zero rows prepended/appended), and the
alpha blend + passthrough-base folds into a host-built coefficient
tensor D, so each core runs a uniform 5-tap sliding-window complex MAC:

  H = [0, 0, spec[1], spec[0], spec[2], ..., spec[T-1], 0, 0]
  D[t,j,2f]   = alpha[t]*cre[t,j,f] + (1-alpha[t])*delta(j==2)
  D[t,j,2f+1] = -alpha[t]*cim[t,j,f]

  per-core (local t):  win[t,j,c] = s[t+j, c]   (s = H slice, c < 192)
    o[t, 2f]   = sum_{j,par} win[t,j,2f+par] * d[t,j,2f+par]
    o[t, 2f+1] = sum_j win[t,j,2f+1]*d[t,j,2f] - sum_j win[t,j,2f]*d[t,j,2f+1]
    o[t, 192:] = s[t+2, 192:]                   (pure DRAM->DRAM copy)

On-core tiling: 125 partitions x 12 frames/partition per tile; the s-load
has a 4-row per-partition overlap so all 5 taps are free-dim shifts.
"""

import numpy as np

NFREQ = 481
NDF = 96
ORDER = 5
W = 2 * NFREQ          # 962 floats per spec row
C = 2 * NDF            # 192 floats of DF bins per row
DW = ORDER * C         # 960 floats per coef row

N_CORES = 8
T_FULL = 60000
TC = T_FULL // N_CORES

# tile geometry
P_DIM = 125
U_FR = 12
UC = 2
PASS_SPLIT = 3

_NC_CACHE = {}


def _build_nc():
    import concourse.bass as bass
    import concourse.bacc as bacc
    import concourse.mybir as mybir
    from concourse.mybir import AluOpType
    from concourse.tile import TileContext

    F32 = mybir.dt.float32
    Tc, P, U = TC, P_DIM, U_FR
    N = P * U
    ntiles = Tc // N
    assert ntiles * N == Tc

    def _view(ap, off, dims):
        return bass.AP(ap.tensor, ap.offset + off, [list(d) for d in dims])

    def _tview(t_ap, off, dims):
        return bass.AP(
            t_ap.tensor, t_ap.offset + off,
            [list(t_ap.ap[0])] + [list(d) for d in dims],
        )

    nc = bacc.Bacc("TRN2", target_bir_lowering=False, debug=False)
    S = nc.dram_tensor("s", [Tc + 4, W], F32, kind="ExternalInput").ap()
    D = nc.dram_tensor("d", [Tc, DW], F32, kind="ExternalInput").ap()
    O = nc.dram_tensor("o", [Tc, W], F32, kind="ExternalOutput").ap()

    with TileContext(nc) as tc:
        with (
            tc.tile_pool(name="sp", bufs=2) as sp,
            tc.tile_pool(name="dp", bufs=3) as dp,
            tc.tile_pool(name="op_", bufs=2) as op_,
            tc.tile_pool(name="wp", bufs=2) as wp,
        ):
            for it in range(ntiles):
                base = it * N

                s_t = sp.tile([P, (U + 4) * C], F32, tag="s")
                nc.sync.dma_start(
                    out=_tview(s_t, 0, [(C, U + 4), (1, C)]),
                    in_=_view(S, base * W, [(U * W, P), (W, U + 4), (1, C)]),
                )

                rows_per = N // PASS_SPLIT
                for ps in range(PASS_SPLIT):
                    r0 = base + ps * rows_per
                    nc.sync.dma_start(
                        out=_view(O, r0 * W + C, [(W, rows_per), (1, W - C)]),
                        in_=_view(S, (r0 + 2) * W + C, [(W, rows_per), (1, W - C)]),
                    )

                o_t = op_.tile([P, U * C], F32, tag="o")

                for uc0 in range(0, U, UC):
                    d_t = dp.tile([P, UC * DW], F32, tag="d")
                    nc.sync.dma_start(
                        out=_tview(d_t, 0, [(DW, UC), (1, DW)]),
                        in_=_view(
                            D, (base + uc0) * DW, [(U * DW, P), (DW, UC), (1, DW)]
                        ),
                    )

                    win = _tview(s_t, uc0 * C, [(C, UC), (C, ORDER), (1, C)])
                    win_e = _tview(s_t, uc0 * C, [(C, UC), (C, ORDER), (2, NDF)])
                    win_o = _tview(s_t, uc0 * C + 1, [(C, UC), (C, ORDER), (2, NDF)])
                    d_full = _tview(d_t, 0, [(DW, UC), (C, ORDER), (1, C)])
                    d_e = _tview(d_t, 0, [(DW, UC), (C, ORDER), (2, NDF)])
                    d_o = _tview(d_t, 1, [(DW, UC), (C, ORDER), (2, NDF)])

                    pq = wp.tile([P, UC * DW], F32, tag="pq")
                    nc.vector.tensor_tensor(
                        _tview(pq, 0, [(DW, UC), (C, ORDER), (1, C)]),
                        win, d_full, AluOpType.mult,
                    )

                    pre = wp.tile([P, UC * ORDER * NDF], F32, tag="pre")
                    pro = wp.tile([P, UC * ORDER * NDF], F32, tag="pro")
                    nc.gpsimd.tensor_tensor(
                        _tview(pre, 0, [(ORDER * NDF, UC), (NDF, ORDER), (1, NDF)]),
                        win_e, d_o, AluOpType.mult,
                    )
                    nc.gpsimd.tensor_tensor(
                        _tview(pro, 0, [(ORDER * NDF, UC), (NDF, ORDER), (1, NDF)]),
                        win_o, d_e, AluOpType.mult,
                    )

                    for v in range(UC):
                        u = uc0 + v
                        nc.vector.tensor_reduce(
                            out=_tview(o_t, u * C, [(2, NDF)]),
                            in_=_tview(pq, v * DW, [(2, NDF), (C, ORDER), (1, 2)]),
                            axis=mybir.AxisListType.XY,
                            op=AluOpType.add,
                        )
                        t_o = wp.tile([P, NDF], F32, tag="t_o")
                        t_e = wp.tile([P, NDF], F32, tag="t_e")
                        nc.vector.tensor_reduce(
                            out=t_o[:],
                            in_=_tview(
                                pro, v * ORDER * NDF, [(1, NDF), (NDF, ORDER)]
                            ),
                            axis=mybir.AxisListType.X,
                            op=AluOpType.add,
                        )
                        nc.vector.tensor_reduce(
                            out=t_e[:],
                            in_=_tview(
                                pre, v * ORDER * NDF, [(1, NDF), (NDF, ORDER)]
                            ),
                            axis=mybir.AxisListType.X,
                            op=AluOpType.add,
                        )
                        nc.vector.tensor_tensor(
                            _tview(o_t, u * C + 1, [(2, NDF)]),
                            t_o[:], t_e[:], AluOpType.subtract,
                        )

                nc.sync.dma_start(
                    out=_view(O, base * W, [(U * W, P), (W, U), (1, C)]),
                    in_=_tview(o_t, 0, [(C, U), (1, C)]),
                )

    nc.compile()
    return nc


def get_nc():
    if "nc" not in _NC_CACHE:
        _NC_CACHE["nc"] = _build_nc()
    return _NC_CACHE["nc"]


def prepare_inputs(spec, coefs, alpha):
    """Host-side shard prep. Returns in_maps for the 8 cores."""
    T = spec.shape[0]
    assert T == T_FULL
    spec_f = np.ascontiguousarray(spec, dtype=np.float32).reshape(T, W)

    H = np.empty((T + 4, W), np.float32)
    H[0:2] = 0.0
    H[2] = spec_f[1]
    H[3] = spec_f[0]
    H[4 : T + 2] = spec_f[2:]
    H[T + 2 :] = 0.0

    a = np.ascontiguousarray(alpha, dtype=np.float32)[:, 0]
    D = np.empty((T, ORDER, NDF, 2), np.float32)
    np.multiply(a[:, None, None], coefs[..., 0], out=D[..., 0])
    np.multiply(-a[:, None, None], coefs[..., 1], out=D[..., 1])
    D[:, 2, :, 0] += (1.0 - a)[:, None]  # base tap: win[t,2] = H[t+2]
    D = D.reshape(T, DW)

    in_maps = [
        {"s": H[c * TC : c * TC + TC + 4], "d": D[c * TC : (c + 1) * TC]}
        for c in range(N_CORES)
    ]
    return in_maps


def run_spmd(in_maps, trace=False, **kwargs):
    from concourse.bass_utils import run_bass_kernel_spmd

    nc = get_nc()
    return run_bass_kernel_spmd(
        nc, in_maps, list(range(N_CORES)), trace=trace, **kwargs
    )


def kernel(spec, coefs, alpha):
    in_maps = prepare_inputs(spec, coefs, alpha)
    res = run_spmd(in_maps).results
    out = np.concatenate([r["o"] for r in res], axis=0)
    return out.reshape(T_FULL, NFREQ, 2)


# revision 3
# speedup vs baseline: 1.6326x; 1.6326x over previous
"""Trainium2 Bass kernel for the DF time-loop module (nn_DfOpTimeLoop).

Strategy
--------
Shard the T=60000 time axis across 8 NeuronCores (7500 frames each, padded
to 7680 = 128*15*4 on-device so tiles use 128 partitions). All the
reference's quirky edge behavior folds into a host-built halo buffer H
(frames 0/1 swapped, zero rows prepended/appended), and the alpha blend +
passthrough-base folds into a host-built coefficient tensor D, so each core
runs a uniform 5-tap sliding-window complex MAC:

  H = [0, 0, spec[1], spec[0], spec[2], ..., spec[T-1], 0, 0, ...]
  D[t,j,2f]   = alpha[t]*cre[t,j,f] + (1-alpha[t])*delta(j==2)
  D[t,j,2f+1] = -alpha[t]*cim[t,j,f]

  per-core (local t):  win[t,j,c] = s[t+j, c]   (s = H slice, c < 192)
    o[t, 2f]   = sum_{j,par} win[t,j,2f+par] * d[t,j,2f+par]
    o[t, 2f+1] = sum_j win[t,j,2f+1]*d[t,j,2f] - sum_j win[t,j,2f]*d[t,j,2f+1]
    o[t, 192:] = s[t+2, 192:]                   (pure DRAM->DRAM copy)

On-core tiling: 128 partitions x 15 frames/partition per tile (partition
count a multiple of 16 so DMA descriptors spray all 16 SDMA engines); the
s-load has a 4-row per-partition overlap so all 5 taps are free-dim shifts.
Loads issue on the Sync HWDGE queue, the passthrough + stores on the Scalar
HWDGE queue, GpSimd takes the imag-path products + subtract, DVE the big
multiply + grouped reduces.
"""

import numpy as np

NFREQ = 481
NDF = 96
ORDER = 5
W = 2 * NFREQ          # 962 floats per spec row
C = 2 * NDF            # 192 floats of DF bins per row
DW = ORDER * C         # 960 floats per coef row

N_CORES = 8
T_FULL = 60000
TC = T_FULL // N_CORES         # real frames per core
TC_PAD = 7680                  # = 128 * 15 * 4, padded on-device frame count

P_DIM = 128
U_FR = 15
UC = 3
PASS_SPLIT = 3

_NC_CACHE = {}


def _build_nc():
    import concourse.bass as bass
    import concourse.bacc as bacc
    import concourse.mybir as mybir
    from concourse.mybir import AluOpType
    from concourse.tile import TileContext

    F32 = mybir.dt.float32
    Tc, P, U = TC_PAD, P_DIM, U_FR
    N = P * U
    ntiles = Tc // N
    assert ntiles * N == Tc

    def _view(ap, off, dims):
        return bass.AP(ap.tensor, ap.offset + off, [list(d) for d in dims])

    def _tview(t_ap, off, dims):
        return bass.AP(
            t_ap.tensor, t_ap.offset + off,
            [list(t_ap.ap[0])] + [list(d) for d in dims],
        )

    nc = bacc.Bacc("TRN2", target_bir_lowering=False, debug=False)
    S = nc.dram_tensor("s", [Tc + 4, W], F32, kind="ExternalInput").ap()
    D = nc.dram_tensor("d", [Tc, DW], F32, kind="ExternalInput").ap()
    O = nc.dram_tensor("o", [Tc, W], F32, kind="ExternalOutput").ap()

    with TileContext(nc) as tc:
        with (
            tc.tile_pool(name="sp", bufs=2) as sp,
            tc.tile_pool(name="dp", bufs=3) as dp,
            tc.tile_pool(name="op_", bufs=2) as op_,
            tc.tile_pool(name="wp", bufs=2) as wp,
        ):
            for it in range(ntiles):
                base = it * N

                s_t = sp.tile([P, (U + 4) * C], F32, tag="s")
                nc.sync.dma_start(
                    out=_tview(s_t, 0, [(C, U + 4), (1, C)]),
                    in_=_view(S, base * W, [(U * W, P), (W, U + 4), (1, C)]),
                )

                rows_per = N // PASS_SPLIT
                for ps in range(PASS_SPLIT):
                    r0 = base + ps * rows_per
                    nc.scalar.dma_start(
                        out=_view(O, r0 * W + C, [(W, rows_per), (1, W - C)]),
                        in_=_view(S, (r0 + 2) * W + C, [(W, rows_per), (1, W - C)]),
                    )

                o_t = op_.tile([P, U * C], F32, tag="o")

                for uc0 in range(0, U, UC):
                    d_t = dp.tile([P, UC * DW], F32, tag="d")
                    nc.sync.dma_start(
                        out=_tview(d_t, 0, [(1, UC * DW)]),
                        in_=_view(
                            D, (base + uc0) * DW, [(U * DW, P), (1, UC * DW)]
                        ),
                    )

                    win = _tview(s_t, uc0 * C, [(C, UC), (C, ORDER), (1, C)])
                    win_e = _tview(s_t, uc0 * C, [(C, UC), (C, ORDER), (2, NDF)])
                    win_o = _tview(
                        s_t, uc0 * C + 1, [(C, UC), (C, ORDER), (2, NDF)]
                    )
                    d_full = _tview(d_t, 0, [(DW, UC), (C, ORDER), (1, C)])
                    d_e = _tview(d_t, 0, [(DW, UC), (C, ORDER), (2, NDF)])
                    d_o = _tview(d_t, 1, [(DW, UC), (C, ORDER), (2, NDF)])

                    pq = wp.tile([P, UC * DW], F32, tag="pq")
                    nc.vector.tensor_tensor(
                        _tview(pq, 0, [(DW, UC), (C, ORDER), (1, C)]),
                        win, d_full, AluOpType.mult,
                    )

                    pre = wp.tile([P, UC * ORDER * NDF], F32, tag="pre")
                    pro = wp.tile([P, UC * ORDER * NDF], F32, tag="pro")
                    nc.gpsimd.tensor_tensor(
                        _tview(
                            pre, 0, [(ORDER * NDF, UC), (NDF, ORDER), (1, NDF)]
                        ),
                        win_e, d_o, AluOpType.mult,
                    )
                    nc.gpsimd.tensor_tensor(
                        _tview(
                            pro, 0, [(ORDER * NDF, UC), (NDF, ORDER), (1, NDF)]
                        ),
                        win_o, d_e, AluOpType.mult,
                    )

                    nc.vector.tensor_reduce(
                        out=_tview(o_t, uc0 * C, [(C, UC), (2, NDF)]),
                        in_=_tview(
                            pq, 0, [(DW, UC), (2, NDF), (C, ORDER), (1, 2)]
                        ),
                        axis=mybir.AxisListType.XY,
                        op=AluOpType.add,
                    )

                    t_o = wp.tile([P, UC * NDF], F32, tag="t_o")
                    t_e = wp.tile([P, UC * NDF], F32, tag="t_e")
                    nc.vector.tensor_reduce(
                        out=_tview(t_o, 0, [(NDF, UC), (1, NDF)]),
                        in_=_tview(
                            pro, 0, [(ORDER * NDF, UC), (1, NDF), (NDF, ORDER)]
                        ),
                        axis=mybir.AxisListType.X,
                        op=AluOpType.add,
                    )
                    nc.vector.tensor_reduce(
                        out=_tview(t_e, 0, [(NDF, UC), (1, NDF)]),
                        in_=_tview(
                            pre, 0, [(ORDER * NDF, UC), (1, NDF), (NDF, ORDER)]
                        ),
                        axis=mybir.AxisListType.X,
                        op=AluOpType.add,
                    )
                    nc.gpsimd.tensor_tensor(
                        _tview(o_t, uc0 * C + 1, [(C, UC), (2, NDF)]),
                        _tview(t_o, 0, [(NDF, UC), (1, NDF)]),
                        _tview(t_e, 0, [(NDF, UC), (1, NDF)]),
                        AluOpType.subtract,
                    )

                nc.scalar.dma_start(
                    out=_view(O, base * W, [(U * W, P), (W, U), (1, C)]),
                    in_=_tview(o_t, 0, [(C, U), (1, C)]),
                )

    nc.compile()
    return nc


def get_nc():
    if "nc" not in _NC_CACHE:
        _NC_CACHE["nc"] = _build_nc()
    return _NC_CACHE["nc"]


def prepare_inputs(spec, coefs, alpha):
    """Host-side shard prep. Returns in_maps for the 8 cores."""
    T = spec.shape[0]
    assert T == T_FULL
    spec_f = np.ascontiguousarray(spec, dtype=np.float32).reshape(T, W)

    h_rows = (N_CORES - 1) * TC + TC_PAD + 4
    H = np.zeros((h_rows, W), np.float32)
    H[2] = spec_f[1]
    H[3] = spec_f[0]
    H[4 : T + 2] = spec_f[2:]

    d_rows = (N_CORES - 1) * TC + TC_PAD
    a = np.ascontiguousarray(alpha, dtype=np.float32)[:, 0]
    D = np.zeros((d_rows, ORDER, NDF, 2), np.float32)
    np.multiply(a[:, None, None], coefs[..., 0], out=D[:T, :, :, 0])
    np.multiply(-a[:, None, None], coefs[..., 1], out=D[:T, :, :, 1])
    D[:T, 2, :, 0] += (1.0 - a)[:, None]  # base tap: win[t,2] = H[t+2]
    D = D.reshape(d_rows, DW)

    in_maps = [
        {
            "s": H[c * TC : c * TC + TC_PAD + 4],
            "d": D[c * TC : c * TC + TC_PAD],
        }
        for c in range(N_CORES)
    ]
    return in_maps


def run_spmd(in_maps, trace=False, **kwargs):
    from concourse.bass_utils import run_bass_kernel_spmd

    nc = get_nc()
    return run_bass_kernel_spmd(
        nc, in_maps, list(range(N_CORES)), trace=trace, **kwargs
    )


def kernel(spec, coefs, alpha):
    in_maps = prepare_inputs(spec, coefs, alpha)
    res = run_spmd(in_maps).results
    out = np.concatenate([r["o"][:TC] for r in res], axis=0)
    return out.reshape(T_FULL, NFREQ, 2)
